# revision 2
# baseline (speedup 1.0000x reference)
"""DeepseekV2 MLA prefill attention on 8 NeuronCores — v2 (head-sharded TP).

Sharding (vLLM-style tensor parallel over heads):
  - down-projections token-parallel (core c owns tokens 256c:256c+256),
    rmsnorm + k_pe rope local, AllGather normalized latents (one ~8.9MB AG).
  - each core up-projects its 4 heads (wq_b/wkv_b column shards) for all
    2048 tokens and runs full causal attention for those heads in S^T
    layout (scores computed transposed: no P transposes, single exp pass,
    denominator via ones-matmul, normalization folded into PV eviction).
  - attention outputs AllGathered in two chunks (overlapping compute),
    wo column-sharded (each core computes out[:, 640c:640c+640]).
Host assembles the full [2048, 5120] output by column concat.

All weights are host-pretiled to [128, ...] partition-major layouts so every
DMA is contiguous per partition.
"""
import sys
import json

sys.path.insert(0, "/opt/trn_rl_repo")

import numpy as np
import ml_dtypes

import concourse.bass as bass
import concourse.mybir as mybir
import concourse.tile as tile
from concourse.bass_utils import run_bass_kernel_spmd

F32 = mybir.dt.float32
F32R = mybir.dt.float32r
F8 = mybir.dt.float8e4
DR_MODE = mybir.MatmulPerfMode.DoubleRow
BF16 = mybir.dt.bfloat16

T = 2048
H = 32
HID = 5120
QL = 1536
KVL = 512
DN = 128
DR = 64
DQK = DN + DR
DV = 128
EPS = 1e-6
SCALING = DQK ** -0.5
NC = 8
OWN = 256          # tokens per core (down-proj shard)
HL = 4             # heads per core
LAT = QL + KVL + DR       # 2112
NLT = 17                  # latent slabs of 128 (last is 64 wide)
HT = HID // 128           # 40 hidden chunks
OC = HID // NC            # 640 own output columns


def legalize_sync_waits(nc):
    """Walrus accepts at most one sync-wait per instruction; split extras onto
    standalone EventSemaphore waits just before (same engine stream)."""
    m = json.loads(nc.to_json_bytes())
    ctr = [0]

    def fresh():
        ctr[0] += 1
        return f"I-lw-{ctr[0]}"

    for f in m["functions"]:
        for bb in f["blocks"]:
            out = []
            for ins in bb["instructions"]:
                si = ins.get("sync_info")
                waits = (si or {}).get("on_wait") or []
                if len(waits) > 1:
                    for w in waits[:-1]:
                        out.append({
                            "debug": ins.get("debug", 0),
                            "engine": ins["engine"],
                            "ins": [], "outs": [],
                            "name": fresh(),
                            "opcode": "EventSemaphore",
                            "sync_info": {"on_update": [], "on_wait": [w]},
                        })
                    si["on_wait"] = waits[-1:]
                out.append(ins)
            bb["instructions"] = out
    nc.m = mybir.module_from_json_bytes(json.dumps(m).encode())
    return nc


def build_bass(sim_mode=False):
    nc = bass.Bass()
    AL = mybir.AluOpType
    AF = mybir.ActivationFunctionType

    dp = nc.declare_dram_parameter
    hidT_d = dp("hidT", [128, HT * OWN], BF16, isOutput=False)
    wdown_d = dp("wdown", [128, NLT * HT * 128], BF16, isOutput=False)
    wqb_d = dp("wqb", [128, 12 * 768], BF16, isOutput=False)
    wkvb_d = dp("wkvb", [128, 4 * 1024], BF16, isOutput=False)
    woh_d = [dp(f"woh{li}", [128, 8 * OC], BF16, isOutput=False)
             for li in range(HL)]
    cosT2_d = dp("cosT2", [128, T], BF16, isOutput=False)
    sinTs2_d = dp("sinTs2", [128, T], BF16, isOutput=False)
    cosk_d = dp("cosk", [DR, OWN], F32, isOutput=False)
    sink_d = dp("sink", [DR, OWN], F32, isOutput=False)
    mask01_d = dp("mask01", [128, 4 * 512], BF16, isOutput=False)
    ones128_d = dp("ones128", [128, 1], F32R, isOutput=False)
    onescol_d = dp("onescol", [128, 1], BF16, isOutput=False)
    onesrow_d = dp("onesrow", [1, 128], F32, isOutput=False)
    outd_d = dp("outd", [OC, T], F32, isOutput=True)

    with tile.TileContext(nc) as tc:
        from contextlib import ExitStack
        st = ExitStack()
        const = st.enter_context(tc.tile_pool(name="const", bufs=1))
        dram = st.enter_context(tc.tile_pool(name="dram", bufs=1, space="DRAM"))

        ones128 = const.tile([128, 1], F32R)
        nc.sync.dma_start(ones128[:], ones128_d[:])
        onescol = const.tile([128, 1], BF16)
        nc.sync.dma_start(onescol[:], onescol_d[:])
        onesrow = const.tile([1, 128], F32)
        nc.sync.dma_start(onesrow[:], onesrow_d[:])
        wqb_sb = const.tile([128, 12, 768], BF16)
        nc.scalar.dma_start(wqb_sb[:], wqb_d[:])
        wkvb_sb = const.tile([128, 4, 1024], BF16)
        nc.scalar.dma_start(wkvb_sb[:], wkvb_d[:])
        kpeT = const.tile([128, T], BF16)
        latTkv = const.tile([128, 4, T], BF16)
        cosT2 = const.tile([128, T], BF16)
        nc.scalar.dma_start(cosT2[:], cosT2_d[:])
        sinTs2 = const.tile([128, T], BF16)
        nc.scalar.dma_start(sinTs2[:], sinTs2_d[:])
        cosk = const.tile([DR, OWN], F32)
        nc.scalar.dma_start(cosk[:], cosk_d[:])
        sink = const.tile([DR, OWN], F32)
        nc.scalar.dma_start(sink[:], sink_d[:])
        mask01 = const.tile([128, 4, 512], BF16)
        nc.scalar.dma_start(mask01[:], mask01_d[:])
        epsc = const.tile([1, 1], F32)
        nc.vector.memset(epsc[:], EPS)
        ln8c = const.tile([1, 1], F32)
        nc.vector.memset(ln8c[:], 2.0794415416798357)

        # DRAM intermediates / collective buffers
        aginq = dram.tile([QL, OWN], BF16)
        aginkv = dram.tile([KVL + 128, OWN], BF16)
        agkvq = dram.tile([NC * QL, OWN], BF16, addr_space="Shared")
        agkvkv = dram.tile([NC * (KVL + 128), OWN], BF16, addr_space="Shared")
        attnsh = [dram.tile([128, T], BF16, name=f"ash{li}")
                  for li in range(HL)]
        agat = [dram.tile([NC * 128, T], BF16, addr_space="Shared",
                          name=f"agat{li}") for li in range(HL)]

        # =========== phase B: down-proj own 256 tokens (out [lat, tok]) =====
        with nc.named_scope("down"):
            pB = ExitStack()
            hidp = pB.enter_context(tc.tile_pool(name="hidp", bufs=1))
            wsl = pB.enter_context(tc.tile_pool(name="wsl", bufs=3))
            latp = pB.enter_context(tc.tile_pool(name="latp", bufs=1))
            sqp = pB.enter_context(tc.tile_pool(name="sqp", bufs=2))
            psB = pB.enter_context(tc.tile_pool(name="psB", bufs=4, space="PSUM"))
            psS = pB.enter_context(tc.tile_pool(name="psS", bufs=2, space="PSUM"))

            hidT = hidp.tile([128, HT, OWN], BF16)
            nc.sync.dma_start(hidT[:], hidT_d[:])

            lat = latp.tile([128, NLT, OWN], F32)
            latn = latp.tile([128, NLT, OWN], BF16)

            ssq_state = {}

            def downproj(lt, grp=None, k=0, nt=0):
                w = 128 if lt < 16 else DR
                wslab = wsl.tile([128, HT, 128], BF16, tag="wslab")
                eng = nc.sync if lt % 2 == 0 else nc.scalar
                eng.dma_start(
                    wslab[:], wdown_d[:, HT * 128 * lt:HT * 128 * (lt + 1)])
                ps = psB.tile([128, OWN], F32, tag="dps")
                for ht in range(HT):
                    nc.tensor.matmul(ps[0:w, :], wslab[:, ht, 0:w],
                                     hidT[:, ht, :],
                                     start=(ht == 0), stop=(ht == HT - 1))
                nc.scalar.copy(lat[0:w, lt, :], ps[0:w, :])
                if grp is not None:
                    # square immediately (DVE), but lag the PE accumulation
                    # matmul one slab so the PE never waits on the chain
                    if k == 0:
                        ssq_state[grp] = psS.tile([1, OWN], F32, tag="ssq",
                                                  name=f"ssq{grp}")
                        ssq_state[(grp, "pend")] = []
                    sq = sqp.tile([128, OWN], F32R, tag="sq", bufs=3)
                    nc.vector.tensor_tensor(out=sq[:], in0=lat[:, lt, :],
                                            in1=lat[:, lt, :], op=AL.mult)
                    pend = ssq_state[(grp, "pend")]
                    pend.append((k, sq))
                    if len(pend) > 1:
                        kk, sqq = pend.pop(0)
                        nc.tensor.matmul(ssq_state[grp][:], ones128[:], sqq[:],
                                         start=(kk == 0), stop=False)
                    if k == nt - 1:
                        kk, sqq = pend.pop(0)
                        nc.tensor.matmul(ssq_state[grp][:], ones128[:], sqq[:],
                                         start=(kk == 0), stop=True)

            def rmsnorm(lt0, nt, L, grp=0):
                ssq = ssq_state[grp]
                f = sqp.tile([1, OWN], F32, tag="f")
                nc.scalar.activation(f[:], ssq[:], AF.Sqrt, bias=epsc[:],
                                     scale=1.0 / L)
                fr = sqp.tile([1, OWN], F32, tag="fr")
                nc.vector.reciprocal(fr[:], f[:])
                fb = psS.tile([128, OWN], F32, tag="fb")
                nc.tensor.matmul(fb[:], onesrow[:], fr[:], start=True, stop=True)
                for k in range(nt):
                    nc.vector.tensor_tensor(out=latn[:, lt0 + k, :],
                                            in0=lat[:, lt0 + k, :], in1=fb[:],
                                            op=AL.mult)

            # kv slabs first so the kv AllGather overlaps the latq down-proj
            for lt in range(12, NLT):
                downproj(lt, grp=1 if lt < 16 else None, k=lt - 12, nt=4)
            rmsnorm(12, 4, KVL, grp=1)
            # rope k_pe (slab 16 rows 0:64), [d, tok] layout, own tokens
            kpsw = sqp.tile([128, OWN], F32, tag="kpsw")
            nc.sync.dma_start(kpsw[0:32, :], lat[32:64, 16, :])
            nc.sync.dma_start(kpsw[32:64, :], lat[0:32, 16, :])
            kpc = sqp.tile([128, OWN], F32, tag="kpc")
            nc.vector.tensor_tensor(out=kpc[0:DR, :], in0=lat[0:DR, 16, :],
                                    in1=cosk[:], op=AL.mult)
            nc.vector.tensor_tensor(out=kpsw[0:DR, :], in0=kpsw[0:DR, :],
                                    in1=sink[:], op=AL.mult)
            nc.vector.tensor_tensor(out=latn[0:DR, 16, :], in0=kpc[0:DR, :],
                                    in1=kpsw[0:DR, :], op=AL.add)
            for k in range(4):
                nc.sync.dma_start(aginkv[128 * k:128 * (k + 1), :],
                                  latn[:, 12 + k, :])
            nc.sync.dma_start(aginkv[KVL:KVL + DR, :], latn[0:DR, 16, :])
            if sim_mode:
                nc.sync.dma_start(agkvkv[0:KVL + 128, :], aginkv[:])
            else:
                nc.gpsimd.collective_compute(
                    "AllGather", AL.bypass, replica_groups=[list(range(NC))],
                    ins=[aginkv.opt()], outs=[agkvkv.opt()])
            for lt in range(12):
                downproj(lt, grp=0, k=lt, nt=12)
            # kv-side latent loads: sync ring reaches here after the last
            # even q wslab; they block nothing that is needed before AG-kv.
            for r in range(NC):
                base = (KVL + 128) * r
                nc.sync.dma_start(kpeT[0:DR, OWN * r:OWN * (r + 1)],
                                  agkvkv[base + KVL:base + KVL + DR, :])
                nc.sync.dma_start(kpeT[64:64 + DR, OWN * r:OWN * (r + 1)],
                                  agkvkv[base + KVL:base + KVL + DR, :])
                nc.sync.dma_start(
                    latTkv[:, :, OWN * r:OWN * (r + 1)],
                    agkvkv[base:base + KVL, :]
                    .rearrange("(a p) t -> p a t", p=128))
            rmsnorm(0, 12, QL, grp=0)
            ssq_state.clear()
            for lt in range(12):
                nc.sync.dma_start(aginq[128 * lt:128 * (lt + 1), :],
                                  latn[:, lt, :])
            if sim_mode:
                nc.sync.dma_start(agkvq[0:QL, :], aginq[:])
            else:
                nc.gpsimd.collective_compute(
                    "AllGather", AL.bypass, replica_groups=[list(range(NC))],
                    ins=[aginq.opt()], outs=[agkvq.opt()])
        pB.close()

        # =========== load latents + weights ===========
        pL = ExitStack()
        latq_p = pL.enter_context(tc.tile_pool(name="latq", bufs=1))
        atp = pL.enter_context(tc.tile_pool(name="atp", bufs=1))
        spp = pL.enter_context(tc.tile_pool(name="spp", bufs=4))
        dnp = pL.enter_context(tc.tile_pool(name="dnp", bufs=2))

        # per-strip latTq tiles: qup strip ts only waits for its 2 rank DMAs
        latTq = [latq_p.tile([128, 12, 512], BF16, name=f"latTq{ts}")
                 for ts in range(4)]

        kTn = [atp.tile([128, T], BF16, name=f"kTn{li}") for li in range(HL)]
        Vsb = atp.tile([128, 16, 512], BF16)
        qTn = [atp.tile([128, T], BF16, name=f"qTn{li}") for li in range(HL)]
        qTp = [atp.tile([128, T], BF16, name=f"qTp{pr}") for pr in range(2)]

        pU = ExitStack()
        psU = pU.enter_context(tc.tile_pool(name="psU", bufs=2, space="PSUM"))
        prp = pU.enter_context(tc.tile_pool(name="prp", bufs=1))

        # ---- kv up-proj: kTn (4 heads) + V (all tok tiles) ----
        with nc.named_scope("kvup"):
            for li in range(HL):
                for ts in range(4):
                    ps = psU.tile([128, 512], F32, tag="ups")
                    for lc in range(4):
                        nc.tensor.matmul(
                            ps[:], wkvb_sb[:, lc, 128 * li:128 * (li + 1)],
                            latTkv[:, lc, 512 * ts:512 * (ts + 1)],
                            start=(lc == 0), stop=(lc == 3))
                    nc.vector.tensor_copy(kTn[li][:, 512 * ts:512 * (ts + 1)],
                                          ps[:])
            for tt in range(16):
                ps = psU.tile([128, 512], F32, tag="ups")
                for lc in range(4):
                    nc.tensor.matmul(ps[:],
                                     latTkv[:, lc, 128 * tt:128 * (tt + 1)],
                                     wkvb_sb[:, lc, 512:1024],
                                     start=(lc == 0), stop=(lc == 3))
                nc.vector.tensor_copy(Vsb[:, tt, :], ps[:])

        # latTq loads: sync ring, after the aginq stores by construction
        for ts in range(4):
            for k in range(2):
                r = 2 * ts + k
                nc.sync.dma_start(
                    latTq[ts][:, :, OWN * k:OWN * (k + 1)],
                    agkvq[QL * r:QL * (r + 1), :]
                    .rearrange("(a p) t -> p a t", p=128))

        # =========== q up-proj + rope, attention per head ============
        psST = pU.enter_context(tc.tile_pool(name="psST", bufs=3, space="PSUM"))
        psPV = pU.enter_context(tc.tile_pool(name="psPV", bufs=2, space="PSUM"))
        psDN = pU.enter_context(tc.tile_pool(name="psDN", bufs=1, space="PSUM"))

        def qup_nope(li):
            for ts in range(4):
                ps = psU.tile([128, 512], F32, tag="ups")
                for lc in range(12):
                    nc.tensor.matmul(
                        ps[:], wqb_sb[:, lc, 128 * li:128 * (li + 1)],
                        latTq[ts][:, lc, :],
                        start=(lc == 0), stop=(lc == 11))
                nc.vector.tensor_copy(qTn[li][:, 512 * ts:512 * (ts + 1)],
                                      ps[:])

        def qup_rope(pr):
            praw = prp.tile([128, T], BF16, tag="praw")
            for ts in range(4):
                ps = psU.tile([128, 512], F32, tag="ups")
                for lc in range(12):
                    nc.tensor.matmul(
                        ps[:], wqb_sb[:, lc, 512 + 128 * pr:640 + 128 * pr],
                        latTq[ts][:, lc, :],
                        start=(lc == 0), stop=(lc == 11))
                nc.vector.tensor_copy(praw[:, 512 * ts:512 * (ts + 1)], ps[:])
            psw = prp.tile([128, T], BF16, tag="psw")
            nc.sync.dma_start(psw[0:32, :], praw[32:64, :])
            nc.sync.dma_start(psw[32:64, :], praw[0:32, :])
            nc.sync.dma_start(psw[64:96, :], praw[96:128, :])
            nc.sync.dma_start(psw[96:128, :], praw[64:96, :])
            nc.vector.tensor_tensor(out=praw[:], in0=praw[:], in1=cosT2[:],
                                    op=AL.mult)
            nc.vector.tensor_tensor(out=psw[:], in0=psw[:], in1=sinTs2[:],
                                    op=AL.mult)
            nc.vector.tensor_tensor(out=qTp[pr][:], in0=praw[:], in1=psw[:],
                                    op=AL.add)

        def attention(li):
            """Full causal attention for local head li; writes attnsh DRAM.
            Lag-1 software pipeline: the PV/den matmuls of chunk i-1 are
            emitted after the score matmuls of chunk i, so the PE never waits
            on the ACT exp of the current chunk."""
            pb = 64 * (li % 2)
            qp = qTp[li // 2]
            for s in range(4):
                qs = slice(512 * s, 512 * (s + 1))
                pvps = psPV.tile([128, 512], F32, tag="pvps",
                                 name=f"pv{li}_{s}")
                denps = psDN.tile([1, 512], F32, tag="denps",
                                  name=f"dn{li}_{s}")
                nchunk = 4 * s + 4

                def scores(i):
                    ks = slice(128 * i, 128 * (i + 1))
                    stps = psST.tile([128, 512], F32, tag="stps")
                    nc.tensor.matmul(stps[:], kTn[li][:, ks], qTn[li][:, qs],
                                     start=True, stop=False)
                    nc.tensor.matmul(stps[:], kpeT[pb:pb + DR, ks],
                                     qp[pb:pb + DR, qs],
                                     start=False, stop=True)
                    pt = spp.tile([128, 512], BF16, tag="pt")
                    nc.scalar.activation(pt[:], stps[:], AF.Exp, scale=SCALING)
                    if i >= 4 * s:
                        nc.vector.tensor_tensor(out=pt[:], in0=pt[:],
                                                in1=mask01[:, i - 4 * s, :],
                                                op=AL.mult)
                    return pt

                def pv_mm(j, pt):
                    nc.tensor.matmul(pvps[:],
                                     Vsb[:, j, 128 * li:128 * (li + 1)],
                                     pt[:], start=(j == 0),
                                     stop=(j == nchunk - 1))

                def den_mm(j, pt):
                    nc.tensor.matmul(denps[:], onescol[:], pt[:],
                                     start=(j == 0), stop=(j == nchunk - 1))

                # PV lags scores by 2 chunks, den by 3: the PE never waits on
                # the ACT exp chain, keeping activity dense (HAM warm clock).
                pts = []
                for i in range(nchunk):
                    pts.append(scores(i))
                    if i >= 2:
                        pv_mm(i - 2, pts[i - 2])
                    if i >= 3:
                        den_mm(i - 3, pts[i - 3])
                for j in range(max(0, nchunk - 2), nchunk):
                    pv_mm(j, pts[j])
                for j in range(max(0, nchunk - 3), nchunk):
                    den_mm(j, pts[j])

                # 1/den via exp(-ln(x)) on ACT: same table set as the exp,
                # avoids the slow single-lane DVE reciprocal.
                dln = dnp.tile([1, 512], F32, tag="dln")
                nc.scalar.activation(dln[:], denps[:], AF.Ln)
                drc = dnp.tile([1, 512], F32, tag="drc")
                nc.scalar.activation(drc[:], dln[:], AF.Exp, scale=-1.0)
                bps = psU.tile([128, 512], F32, tag="ups", name=f"b{li}_{s}")
                nc.tensor.matmul(bps[:], onesrow[:], drc[:], start=True,
                                 stop=True)
                denb = spp.tile([128, 512], F32, tag="denb")
                nc.vector.tensor_copy(denb[:], bps[:])
                ao = spp.tile([128, 512], BF16, tag="ao")
                nc.vector.tensor_tensor(out=ao[:], in0=pvps[:], in1=denb[:],
                                        op=AL.mult)
                nc.sync.dma_start(attnsh[li][:, qs], ao[:])

        def ag_attn(li):
            if sim_mode:
                nc.sync.dma_start(agat[li][0:128, :], attnsh[li][:])
            else:
                nc.gpsimd.collective_compute(
                    "AllGather", AL.bypass, replica_groups=[list(range(NC))],
                    ins=[attnsh[li].opt()], outs=[agat[li].opt()])

        with nc.named_scope("attn"):
            qup_nope(0)
            qup_rope(0)
            qup_nope(1)
            attention(0)
            ag_attn(0)
            qup_nope(2)
            attention(1)
            ag_attn(1)
            qup_nope(3)
            qup_rope(1)
            attention(2)
            ag_attn(2)
            attention(3)
            ag_attn(3)
        pU.close()
        pL.close()

        # =========== wo (column shard, 640 own cols) ===========
        with nc.named_scope("wo"):
            pW = ExitStack()
            wop = pW.enter_context(tc.tile_pool(name="wop", bufs=1))
            chp = pW.enter_context(tc.tile_pool(name="chp", bufs=4))
            oev = pW.enter_context(tc.tile_pool(name="oev", bufs=3))
            psW = pW.enter_context(tc.tile_pool(name="psW", bufs=1,
                                                space="PSUM"))

            woh_sb = []
            for li in range(HL):
                w = wop.tile([128, 8, OC], BF16, name=f"woh{li}")
                nc.scalar.dma_start(w[:], woh_d[li][:])
                woh_sb.append(w)

            for tq in range(4):
                qs = slice(512 * tq, 512 * (tq + 1))
                pss = [psW.tile([128, 512], F32, tag=f"wps{hc}",
                                name=f"w{tq}_{hc}") for hc in range(5)]
                for grp in range(HL):
                    agd, wsb = agat[grp], woh_sb[grp]
                    for kc in range(8):
                        ch = chp.tile([128, 512], BF16, tag="ch")
                        # grp3 chunks wait on the last AllGather: keep them
                        # off the sync ring so later loads aren't blocked
                        eng = nc.gpsimd if grp == HL - 1 else nc.sync
                        eng.dma_start(ch[:],
                                      agd[128 * kc:128 * (kc + 1), qs])
                        first = (grp == 0 and kc == 0)
                        last = (grp == HL - 1 and kc == 7)
                        for hc in range(5):
                            nc.tensor.matmul(pss[hc][:], wsb[:, kc,
                                             128 * hc:128 * (hc + 1)],
                                             ch[:], start=first, stop=last)
                for hc in range(5):
                    ev = oev.tile([128, 512], F32, tag="oevt")
                    nc.scalar.copy(ev[:], pss[hc][:])
                    nc.sync.dma_start(outd_d[128 * hc:128 * (hc + 1), qs],
                                      ev[:])
            pW.close()
        st.close()

    nc.finalize()
    legalize_sync_waits(nc)
    return nc


_DEINT = np.array([2 * r if r < 32 else 2 * r - 63 for r in range(DR)])


def _host_prep(inputs):
    f32 = np.float32
    bf16 = ml_dtypes.bfloat16
    f8 = mybir.dt.np(mybir.dt.float8e4)
    hs = np.asarray(inputs["hidden_states"], f32)
    cos = np.asarray(inputs["cos"], f32).reshape(T, DR)
    sin = np.asarray(inputs["sin"], f32).reshape(T, DR)
    wq_a = np.asarray(inputs["wq_a"], f32)
    q_ln = np.asarray(inputs["q_a_ln_w"], f32)
    wq_b = np.asarray(inputs["wq_b"], f32) * q_ln[:, None]
    wkv_a = np.asarray(inputs["wkv_a"], f32)
    kv_ln = np.asarray(inputs["kv_a_ln_w"], f32)
    wkv_b = np.asarray(inputs["wkv_b"], f32) * kv_ln[:, None]
    wo = np.asarray(inputs["wo"], f32)

    # ---- shared (replicated) prep ----
    wkvap = wkv_a.copy()
    wkvap[:, KVL:] = wkv_a[:, KVL:][:, _DEINT]
    wdown = np.concatenate([wq_a, wkvap], axis=1)          # [HID, 2112]
    wdown_pad = np.zeros((HID, NLT * 128), f32)
    wdown_pad[:, 0:LAT] = wdown
    # pretile [p, lt, a, c]
    wdown_t = np.ascontiguousarray(
        wdown_pad.reshape(HT, 128, NLT, 128).transpose(1, 2, 0, 3)
    ).reshape(128, NLT * HT * 128).astype(bf16)

    cosT = cos.T
    sinT = sin.T
    sinTs = sinT.copy()
    sinTs[0:32] = -sinT[0:32]
    cosT2f = np.ascontiguousarray(np.concatenate([cosT, cosT], axis=0))
    sinTs2f = np.ascontiguousarray(np.concatenate([sinTs, sinTs], axis=0))
    cosT2 = cosT2f.astype(bf16)
    sinTs2 = sinTs2f.astype(bf16)

    mask01 = np.zeros((128, 4, 512), f32)
    pp = np.arange(128)[:, None]
    qq = np.arange(512)[None, :]
    for d in range(4):
        mask01[:, d, :] = (pp + 128 * d <= qq)
    mask01 = mask01.reshape(128, 4 * 512).astype(bf16)

    ones128 = np.ones((128, 1), f32)
    onescol_b = np.ones((128, 1), f32).astype(bf16)
    onesrow = np.ones((1, 128), f32)

    wor = wo.reshape(NC, HL, DN, HID)
    woh_all = [wor[:, li].reshape(NC * DN, HID) for li in range(HL)]

    in_maps = []
    for c in range(NC):
        hsT = hs[OWN * c:OWN * (c + 1)].T                  # [HID, 256]
        hidT_t = np.ascontiguousarray(
            hsT.reshape(HT, 128, OWN).transpose(1, 0, 2)
        ).reshape(128, HT * OWN).astype(bf16)

        # wqb columns: 4 heads nope (512) + 2 rope pairs (256), deint folded
        wqbcols = np.zeros((QL, 768), f32)
        for li in range(HL):
            h = HL * c + li
            wqbcols[:, 128 * li:128 * (li + 1)] = wq_b[:, DQK * h:DQK * h + DN]
        for pr in range(2):
            for k in range(2):
                h = HL * c + 2 * pr + k
                pe = wq_b[:, DQK * h + DN:DQK * (h + 1)][:, _DEINT]
                wqbcols[:, 512 + 128 * pr + 64 * k:
                        512 + 128 * pr + 64 * (k + 1)] = pe
        wqb_t = np.ascontiguousarray(
            wqbcols.reshape(12, 128, 768).transpose(1, 0, 2)
        ).reshape(128, 12 * 768).astype(bf16)

        wkvbcols = np.zeros((KVL, 1024), f32)
        for li in range(HL):
            h = HL * c + li
            wkvbcols[:, 128 * li:128 * (li + 1)] = \
                wkv_b[:, 256 * h:256 * h + DN]
            wkvbcols[:, 512 + 128 * li:512 + 128 * (li + 1)] = \
                wkv_b[:, 256 * h + DN:256 * (h + 1)]
        wkvb_t = np.ascontiguousarray(
            wkvbcols.reshape(4, 128, 1024).transpose(1, 0, 2)
        ).reshape(128, 4 * 1024).astype(bf16)

        own = slice(OC * c, OC * (c + 1))
        woh_t = {}
        for li in range(HL):
            woh_t[f"woh{li}"] = np.ascontiguousarray(
                woh_all[li][:, own].reshape(8, 128, OC).transpose(1, 0, 2)
            ).reshape(128, 8 * OC).astype(bf16)

        in_maps.append({
            "hidT": hidT_t,
            "wdown": wdown_t,
            "wqb": wqb_t,
            "wkvb": wkvb_t,
            **woh_t,
            "cosT2": cosT2,
            "sinTs2": sinTs2,
            "cosk": np.ascontiguousarray(cosT2f[0:DR, OWN * c:OWN * (c + 1)]),
            "sink": np.ascontiguousarray(sinTs2f[0:DR, OWN * c:OWN * (c + 1)]),
            "mask01": mask01,
            "ones128": ones128,
            "onescol": onescol_b,
            "onesrow": onesrow,
        })
    return in_maps


_NC_CACHE = None


def _get_nc():
    global _NC_CACHE
    if _NC_CACHE is None:
        _NC_CACHE = build_bass()
    return _NC_CACHE


def run(inputs, trace=False):
    nc = _get_nc()
    in_maps = _host_prep(inputs)
    res = run_bass_kernel_spmd(nc, in_maps, list(range(NC)), trace=trace)
    out = np.empty((T, HID), np.float32)
    for c in range(NC):
        out[:, OC * c:OC * (c + 1)] = res.results[c]["outd"].T
    return out, res


def kernel(**inputs):
    out, _ = run(inputs, trace=False)
    return out
